# revision 1
# baseline (speedup 1.0000x reference)
"""Trainium2 Bass kernel for the MemoryReader (retrieval-knn) module.

Math (per batch b):
    a[m]     = sum_ck mk[ck, m]^2
    logits   = (2 * mk^T qk - a) / sqrt(CK)        # [THW, NQ]
    aff      = softmax(logits, axis=THW)
    out      = mv @ aff                            # [CV, NQ]

Shapes: B=4, CK=64, T=8, H=30, W=54 (THW=12960, NQ=1620), CV=512.

Sharding: 8 cores = (B=4) x (NQ halves of 810).  Softmax is over THW,
which every core owns fully, so no cross-core reduction is needed.

Device-side trick: the squared-norm term is folded into the score
matmul by augmenting the contraction dim to K=128:
    lhsT' = [mk ; mk^2]  (host-prepared, [128, THW])
    rhs'  = [qk ; -0.5 ]  (host-prepared, [128, 810])
    psum  = mk.qk - a/2  ->  logits = 0.25 * psum  (ACT scale)
Scores never need a softmax max-subtraction: with these inputs logits
are in [-27, 4] and exp sums stay < 300, comfortably inside fp32.

Matmuls run in float32r (full PE rate; ~1e-3 rel err).  The readout
contracts over THW with mv host-transposed to [THW, CV] so every DMA
is a contiguous 2KB-per-partition stream.
"""

import os
import sys

import numpy as np

for _p in ("/opt/trn_rl_repo",):
    if _p not in sys.path and os.path.isdir(_p):
        sys.path.insert(0, _p)

B, CK, T, H, W = 4, 64, 8, 30, 54
CV = 512
THW = T * H * W          # 12960
NQ = H * W               # 1620
QH = NQ // 2             # 810   per-core query half
QBLKS = [(0, 406), (406, 404)]  # even widths/offsets (f32r needs N%2==0)
QBMAX = 406
P = 128
M_TILES = [(m0, min(P, THW - m0)) for m0 in range(0, THW, P)]  # 101x128 + 1x32
MKQ_CHUNK = 4 * P        # columns per mkq prefetch chunk

_PROGRAM = None


def _build_program():
    import concourse.mybir as mybir
    import concourse.tile as tile
    from concourse import bacc

    f32 = mybir.dt.float32
    f32r = mybir.dt.float32r
    Exp = mybir.ActivationFunctionType.Exp

    nc = bacc.Bacc(
        "TRN2",
        target_bir_lowering=False,
        debug=False,
        enable_asserts=False,
        num_devices=8,
    )

    mkq = nc.dram_tensor("mkq", [P, THW], f32r, kind="ExternalInput").ap()
    qkc = nc.dram_tensor("qkc", [P, QH], f32r, kind="ExternalInput").ap()
    mvt = nc.dram_tensor("mvt", [THW, CV], f32r, kind="ExternalInput").ap()
    out = nc.dram_tensor("out", [CV, QH], f32, kind="ExternalOutput").ap()

    with tile.TileContext(nc) as tc:
        with (
            tc.tile_pool(name="const", bufs=1) as cpool,
            tc.tile_pool(name="mvt", bufs=4) as mvpool,
            tc.tile_pool(name="exp", bufs=3) as expool,
            tc.tile_pool(name="vec", bufs=2) as vpool,
            tc.tile_pool(name="outp", bufs=4) as opool,
            tc.tile_pool(name="score_ps", bufs=2, space="PSUM") as spspool,
            tc.tile_pool(name="acc_ps", bufs=1, space="PSUM") as apspool,
            tc.tile_pool(name="misc_ps", bufs=1, space="PSUM") as mpspool,
        ):
            mkq_sb = cpool.tile([P, THW], f32r, tag="mkq", name="mkq")
            for c0 in range(0, THW, MKQ_CHUNK):
                c1 = min(c0 + MKQ_CHUNK, THW)
                nc.sync.dma_start(out=mkq_sb[:, c0:c1], in_=mkq[:, c0:c1])
            qkc_sb = cpool.tile([P, QH], f32r, tag="qkc", name="qkc")
            nc.sync.dma_start(out=qkc_sb[:], in_=qkc[:])
            ones_col = cpool.tile([P, 1], f32, tag="ones_col", name="ones_col")
            nc.vector.memset(ones_col[:], 1.0)
            ones_row = cpool.tile([1, P], f32, tag="ones_row", name="ones_row")
            nc.vector.memset(ones_row[:], 1.0)

            for q0, nq in QBLKS:
                accs = [apspool.tile([P, nq], f32, tag=f"acc{c}", name=f"acc{c}") for c in range(4)]
                den = vpool.tile([P, nq], f32, tag="den", name="den")
                nc.vector.memset(den[:], 0.0)

                for mi, (m0, mp) in enumerate(M_TILES):
                    mv_t = mvpool.tile([P, CV], f32r, tag="mvt", name="mvt")
                    nc.sync.dma_start(out=mv_t[:mp, :], in_=mvt[m0 : m0 + mp, :])
                    score = spspool.tile([P, nq], f32, tag="score", name="score")
                    nc.tensor.matmul(
                        score[:mp, :],
                        lhsT=mkq_sb[:, m0 : m0 + mp],
                        rhs=qkc_sb[:, q0 : q0 + nq],
                        start=True,
                        stop=True,
                    )
                    ex = expool.tile([P, nq], f32r, tag="exp", name="exp")
                    nc.scalar.activation(
                        ex[:mp, :], score[:mp, :], Exp, bias=0.0, scale=0.25
                    )
                    nc.vector.tensor_add(den[:mp, :], den[:mp, :], ex[:mp, :].bitcast(f32))
                    for c in range(4):
                        nc.tensor.matmul(
                            accs[c][:, :],
                            lhsT=mv_t[:mp, c * P : (c + 1) * P],
                            rhs=ex[:mp, :],
                            start=(mi == 0),
                            stop=(mi == len(M_TILES) - 1),
                        )

                den_sum = mpspool.tile([1, nq], f32, tag="den_sum", name="den_sum")
                nc.tensor.matmul(
                    den_sum[:], lhsT=ones_col[:], rhs=den[:], start=True, stop=True
                )
                recip = vpool.tile([1, nq], f32, tag="recip", name="recip")
                nc.vector.reciprocal(recip[:], den_sum[:])
                bcast_ps = mpspool.tile([P, nq], f32, tag="bcast_ps", name="bcast_ps")
                nc.tensor.matmul(
                    bcast_ps[:], lhsT=ones_row[:], rhs=recip[:], start=True, stop=True
                )
                bcast_sb = vpool.tile([P, nq], f32, tag="bcast_sb", name="bcast_sb")
                nc.vector.tensor_copy(bcast_sb[:], bcast_ps[:])
                for c in range(4):
                    o = opool.tile([P, nq], f32, tag="out", name="out")
                    nc.vector.tensor_mul(o[:], accs[c][:, :], bcast_sb[:])
                    nc.sync.dma_start(
                        out=out[c * P : (c + 1) * P, q0 : q0 + nq], in_=o[:]
                    )

    nc.compile()
    return nc


def _get_program():
    global _PROGRAM
    if _PROGRAM is None:
        _PROGRAM = _build_program()
    return _PROGRAM


def _make_in_maps(mk, qk, mv):
    mkf = np.ascontiguousarray(mk.reshape(B, CK, THW), dtype=np.float32)
    qkf = np.ascontiguousarray(qk.reshape(B, CK, NQ), dtype=np.float32)
    mvf = mv.reshape(B, CV, THW)

    in_maps = []
    for b in range(B):
        mkq_b = np.concatenate([mkf[b], mkf[b] * mkf[b]], axis=0)  # [128, THW]
        mvt_b = np.ascontiguousarray(mvf[b].T, dtype=np.float32)   # [THW, CV]
        for h in range(2):
            qkc_b = np.concatenate(
                [
                    qkf[b][:, h * QH : (h + 1) * QH],
                    np.full((CK, QH), -0.5, dtype=np.float32),
                ],
                axis=0,
            )  # [128, QH]
            in_maps.append(
                {
                    "mkq": mkq_b,
                    "qkc": np.ascontiguousarray(qkc_b),
                    "mvt": mvt_b,
                }
            )
    return in_maps


def kernel(mk, qk, mv, _trace=False, _results_out=None):
    from concourse import bass_utils

    nc = _get_program()
    in_maps = _make_in_maps(np.asarray(mk), np.asarray(qk), np.asarray(mv))
    res = bass_utils.run_bass_kernel_spmd(
        nc, in_maps, core_ids=list(range(8)), trace=_trace
    )
    if _results_out is not None:
        _results_out.append(res)

    full = np.empty((B, CV, NQ), dtype=np.float32)
    for b in range(B):
        for h in range(2):
            full[b][:, h * QH : (h + 1) * QH] = res.results[2 * b + h]["out"]
    return full.reshape(B, CV, H, W)

